# revision 8
# baseline (speedup 1.0000x reference)
"""Trainium2 Bass kernel for the masked-MSE actor-critic criterion.

Problem: inputs sample_seq/sample_value/sample_reward, all [65536, 256].
  mask[i, j] = 1 iff no zero appears in sample_seq[i, :j]  (prefix property)
  loss       = sum((reward-value)^2 * mask) / sum(mask)
  returns (loss, mean(reward-value), mean(reward))

Strategy (pure data-parallel over 8 NeuronCores):
  - Host shards the batch dim 8 ways and TRANSPOSES each shard to [S=256, 8192]
    so the sequence dim lies along SBUF partitions (2 blocks of 128).
  - seq is sent as uint8 (values 0..19, lossless), reward/value as bf16.
  - On device, per column-tile of R=512 batch rows:
      g = (seq == 0)                            (VectorE, bf16 0/1)
      C = Tri^T @ g (+ block0 total for block1) (TensorE, prefix zero-counts)
      mask = relu(1 - C)                        (ScalarE, PSUM -> SBUF bf16)
      d = r - v; dk = d*mask; dm = d*dk         (VectorE, bf16 2x mode)
      stats[0..3] += ones^T @ {dm, mask, r, d}  (TensorE, PSUM accumulate)
  - Final: reduce stats [4, R] -> [4, 1], DMA out. Host sums the 8 cores'
    partial stats and forms the 3 outputs.
"""

import numpy as np

B, S = 65536, 256
N_CORES = 8
P = 128
COLS = B // N_CORES  # 8192 columns (batch rows) per core
R = 512              # batch columns per compute tile (one PSUM bank)

_cache = {}


def build_nc(cols, r=R):
    from concourse import bacc, tile, mybir

    dt = mybir.dt
    ntiles = cols // r
    assert cols % r == 0

    nc = bacc.Bacc("TRN2", target_bir_lowering=False, debug=False,
                   num_devices=N_CORES)

    seq_d = nc.declare_dram_parameter("seq", [S, cols], dt.uint8, isOutput=False)
    rew_d = nc.declare_dram_parameter("rew", [S, cols], dt.bfloat16, isOutput=False)
    val_d = nc.declare_dram_parameter("val", [S, cols], dt.bfloat16, isOutput=False)
    tri_d = nc.declare_dram_parameter("tri", [P, P], dt.bfloat16, isOutput=False)
    ones_d = nc.declare_dram_parameter("ones", [P, 1], dt.bfloat16, isOutput=False)
    onesk1_d = nc.declare_dram_parameter("onesk1", [1, P], dt.bfloat16, isOutput=False)
    out_d = nc.declare_dram_parameter("out", [1, 4], dt.float32, isOutput=True)

    with tile.TileContext(nc) as tc:
        with (
            tc.tile_pool(name="const", bufs=1) as constp,
            tc.tile_pool(name="io", bufs=3) as iop,
            tc.tile_pool(name="mid", bufs=3) as midp,
            tc.tile_pool(name="cpsum", bufs=1, space="PSUM") as cpsump,
            tc.tile_pool(name="tpsum", bufs=2, space="PSUM") as tpsump,
            tc.tile_pool(name="spsum", bufs=1, space="PSUM") as spsump,
            tc.tile_pool(name="outp", bufs=1) as outp,
        ):
            tri_t = constp.tile([P, P], dt.bfloat16)
            nc.sync.dma_start(tri_t[:], tri_d[:])
            ones_t = constp.tile([P, 1], dt.bfloat16)
            nc.sync.dma_start(ones_t[:], ones_d[:])
            onesk1_t = constp.tile([1, P], dt.bfloat16)
            nc.sync.dma_start(onesk1_t[:], onesk1_d[:])

            # stats segments (partition 0, one PSUM bank each):
            #   0 = sum(d^2*mask), 1 = sum(mask), 2 = sum(r), 3 = sum(d)
            stats = spsump.tile([1, 4 * r], dt.float32)

            for t in range(ntiles):
                c0 = t * r
                first = t == 0
                last = t == ntiles - 1

                seq0 = iop.tile([P, r], dt.uint8, tag="seq0")
                seq1 = iop.tile([P, r], dt.uint8, tag="seq1")
                r0 = iop.tile([P, r], dt.bfloat16, tag="r0")
                r1 = iop.tile([P, r], dt.bfloat16, tag="r1")
                v0 = iop.tile([P, r], dt.bfloat16, tag="v0")
                v1 = iop.tile([P, r], dt.bfloat16, tag="v1")
                nc.sync.dma_start(seq0[:], seq_d[0:P, c0:c0 + r])
                nc.sync.dma_start(seq1[:], seq_d[P:S, c0:c0 + r])
                nc.sync.dma_start(r0[:], rew_d[0:P, c0:c0 + r])
                nc.sync.dma_start(r1[:], rew_d[P:S, c0:c0 + r])
                nc.sync.dma_start(v0[:], val_d[0:P, c0:c0 + r])
                nc.sync.dma_start(v1[:], val_d[P:S, c0:c0 + r])

                g0 = midp.tile([P, r], dt.bfloat16, tag="g0")
                g1 = midp.tile([P, r], dt.bfloat16, tag="g1")
                nc.vector.tensor_scalar(g0[:], seq0[:], 0.0, None,
                                        mybir.AluOpType.is_equal)
                nc.vector.tensor_scalar(g1[:], seq1[:], 0.0, None,
                                        mybir.AluOpType.is_equal)

                c0p = cpsump.tile([P, r], dt.float32, tag="c0p")
                c1p = cpsump.tile([P, r], dt.float32, tag="c1p")
                tot0p = tpsump.tile([1, r], dt.float32, tag="tot0p")
                nc.tensor.matmul(c0p[:], tri_t[:], g0[:])
                nc.tensor.matmul(tot0p[:], ones_t[:], g0[:])
                tot0s = midp.tile([1, r], dt.bfloat16, tag="tot0s")
                nc.scalar.copy(tot0s[:], tot0p[:])
                nc.tensor.matmul(c1p[:], tri_t[:], g1[:], start=True, stop=False)
                nc.tensor.matmul(c1p[:], onesk1_t[:], tot0s[:],
                                 start=False, stop=True)

                mask0 = midp.tile([P, r], dt.bfloat16, tag="mask0")
                mask1 = midp.tile([P, r], dt.bfloat16, tag="mask1")
                nc.scalar.activation(mask0[:], c0p[:],
                                     mybir.ActivationFunctionType.Relu,
                                     bias=1.0, scale=-1.0)
                nc.scalar.activation(mask1[:], c1p[:],
                                     mybir.ActivationFunctionType.Relu,
                                     bias=1.0, scale=-1.0)

                d0 = midp.tile([P, r], dt.bfloat16, tag="d0")
                d1 = midp.tile([P, r], dt.bfloat16, tag="d1")
                nc.vector.tensor_tensor(d0[:], r0[:], v0[:],
                                        mybir.AluOpType.subtract)
                nc.vector.tensor_tensor(d1[:], r1[:], v1[:],
                                        mybir.AluOpType.subtract)
                dk0 = midp.tile([P, r], dt.bfloat16, tag="dk0")
                dk1 = midp.tile([P, r], dt.bfloat16, tag="dk1")
                nc.vector.tensor_tensor(dk0[:], d0[:], mask0[:],
                                        mybir.AluOpType.mult)
                nc.vector.tensor_tensor(dk1[:], d1[:], mask1[:],
                                        mybir.AluOpType.mult)
                dm0 = midp.tile([P, r], dt.bfloat16, tag="dm0")
                dm1 = midp.tile([P, r], dt.bfloat16, tag="dm1")
                nc.vector.tensor_tensor(dm0[:], d0[:], dk0[:],
                                        mybir.AluOpType.mult)
                nc.vector.tensor_tensor(dm1[:], d1[:], dk1[:],
                                        mybir.AluOpType.mult)

                # stats accumulation: row 0 dm, row 1 mask, row 2 r, row 3 d
                for row, (t0, t1) in enumerate(
                    [(dm0, dm1), (mask0, mask1), (r0, r1), (d0, d1)]
                ):
                    seg = stats[0:1, row * r:(row + 1) * r]
                    nc.tensor.matmul(seg, ones_t[:], t0[:],
                                     start=first, stop=False,
                                     skip_group_check=True)
                    nc.tensor.matmul(seg, ones_t[:], t1[:],
                                     start=False, stop=last,
                                     skip_group_check=True)

            outs = outp.tile([1, 4], dt.float32)
            stats3 = stats[:].rearrange("p (s r) -> p s r", s=4)
            nc.vector.reduce_sum(outs[:].rearrange("p (s o) -> p s o", o=1),
                                 stats3, axis=mybir.AxisListType.X)
            nc.sync.dma_start(out_d[:], outs[:])

    nc.compile()
    return nc


def make_consts():
    import ml_dtypes
    bf16 = ml_dtypes.bfloat16
    # tri[k, j] = 1 if k < j  (strictly-lower prefix: C[j] = # zeros before j)
    tri = np.triu(np.ones((P, P), dtype=np.float32), 1).astype(bf16)
    ones = np.ones((P, 1), dtype=bf16)
    onesk1 = np.ones((1, P), dtype=bf16)
    return tri, ones, onesk1


def prep_shards(sample_seq, sample_value, sample_reward):
    """Host-side shard prep: batch-shard 8 ways, transpose to [S, cols]."""
    import ml_dtypes
    bf16 = ml_dtypes.bfloat16
    seq_u8 = np.asarray(sample_seq).astype(np.uint8)      # values in [0, 20)
    rew_bf = np.asarray(sample_reward).astype(bf16)
    val_bf = np.asarray(sample_value).astype(bf16)

    tri, ones, onesk1 = make_consts()
    in_maps = []
    for c in range(N_CORES):
        lo, hi = c * COLS, (c + 1) * COLS
        in_maps.append({
            "seq": np.ascontiguousarray(seq_u8[lo:hi].T),
            "rew": np.ascontiguousarray(rew_bf[lo:hi].T),
            "val": np.ascontiguousarray(val_bf[lo:hi].T),
            "tri": tri,
            "ones": ones,
            "onesk1": onesk1,
        })
    return in_maps


def combine(stats_per_core):
    """stats_per_core: [n_cores, 4] partial sums -> (3,) f32 output."""
    tot = np.asarray(stats_per_core, dtype=np.float64).sum(axis=0)
    n = float(B) * float(S)
    loss = tot[0] / tot[1]
    dmean = tot[3] / n
    rmean = tot[2] / n
    return np.array([loss, dmean, rmean], dtype=np.float32)


def run(sample_seq, sample_value, sample_reward, trace=False, **kwargs):
    from concourse.bass_utils import run_bass_kernel_spmd

    if "nc" not in _cache:
        _cache["nc"] = build_nc(COLS)
    nc = _cache["nc"]

    in_maps = prep_shards(sample_seq, sample_value, sample_reward)
    res = run_bass_kernel_spmd(nc, in_maps, core_ids=list(range(N_CORES)),
                               trace=trace, **kwargs)
    stats = np.stack([res.results[i]["out"][0, :] for i in range(N_CORES)])
    return combine(stats), res


def kernel(sample_seq, sample_value, sample_reward):
    out, _ = run(sample_seq, sample_value, sample_reward)
    return out
